# revision 47
# baseline (speedup 1.0000x reference)
"""GraphSAGE 2-layer GNN + MLP head on 8 Trainium2 NeuronCores (v4).

Strategy (dst-sharded, dense-adjacency scatter, fp8 DoubleRow):
  - Destination nodes sharded across 8 cores; node index space padded to
    1280 slots/core (10240 global slots = 80 full 128-chunks) so every
    matmul chunk is full and fp8 DoubleRow pairs align.
  - Scatter-mean collapses to  aggr = (relu(X W))^T A_mean  with
    A_mean[src,dst] = edge_count/deg(dst) in fp8 e4m3 (mean folded in
    host-side), resident in SBUF and reused by both layers.
    Scatter matmuls run in fp8 DoubleRow (K=256 per instruction).
  - A is stored partition-major in DRAM ([128, 80, 1250]); 10 slab
    dma_starts with 12.5KB/partition descriptors saturate HBM; y0 and
    the layer-0 scatter are emitted interleaved per slab.
  - y0/y1/z2 copy/activation work is batched 4 chunks per PSUM bank
    (512B quarter-bank matmul outputs, one wide relu per batch) to avoid
    serializing on per-chunk scalar ops.
  - Inter-layer AllGather of fp8 y1 split in two (6+4 chunks): the
    second collective overlaps the first half's scatter matmuls.
  - Row L2-norm: ones-matmul partition reduction, scalar Sqrt, DVE fast
    reciprocal - all partition-parallel.
  - log_softmax: second post_mp matmul emits node-major logits directly;
    single big Exp + tensor_reduce keeps act-table loads at 2.
"""

import numpy as np
import ml_dtypes

import concourse.bacc as bacc
import concourse.mybir as mybir
from concourse import tile
from concourse.bass_utils import run_bass_kernel_spmd

N_NODES = 10000
N_CORES = 8
SHARD = N_NODES // N_CORES   # 1250 real dst nodes per core
P = 128
JC = 10                      # local 128-chunks per core (1280 slots)
SLOTS = JC * P               # 1280 padded slots per core
G = N_CORES * SLOTS          # 10240 padded global slots
KC = G // P                  # 80 src chunks
KP = KC // 2                 # 40 DoubleRow pairs
F = 128
FOUT = 64
NCHUNKS = [(0, 512), (512, 512), (1024, SHARD - 1024)]
JA = 6                       # allgather half A: local chunks 0..5
ASLAB = 8                    # a8 chunks per dma slab

FP8 = mybir.dt.float8e4
BF16 = mybir.dt.bfloat16
F32 = mybir.dt.float32
DR = mybir.MatmulPerfMode.DoubleRow
AF = mybir.ActivationFunctionType

NP_FP8 = ml_dtypes.float8_e4m3
NP_BF16 = ml_dtypes.bfloat16


def _jc(j):
    """real node count in local chunk j (last chunk is partial: 98)."""
    return min(P, SHARD - j * P)


def build():
    nc = bacc.Bacc("TRN2", target_bir_lowering=False, debug=False,
                   num_devices=N_CORES)

    # ---- external I/O ----
    xt_d = nc.declare_dram_parameter("xt", [P, G], FP8, isOutput=False)
    xtsh_d = nc.declare_dram_parameter("xt_sh", [P, SLOTS], BF16, isOutput=False)
    a8_d = nc.declare_dram_parameter("a8", [P, KC, SHARD], FP8, isOutput=False)
    lin_w0_d = nc.declare_dram_parameter("lin_w0", [F, F], FP8, isOutput=False)
    lin_w1_d = nc.declare_dram_parameter("lin_w1", [F, F], BF16, isOutput=False)
    agg_w0_d = nc.declare_dram_parameter("agg_w0", [2 * F, F], BF16, isOutput=False)
    agg_w1_d = nc.declare_dram_parameter("agg_w1", [2 * F, F], BF16, isOutput=False)
    mp_w12_d = nc.declare_dram_parameter("mp_w12", [F, FOUT], BF16,
                                         isOutput=False)
    out_d = nc.declare_dram_parameter("out", [SHARD, FOUT], F32, isOutput=True)

    # internal DRAM for the split inter-layer AllGather
    warm_in_d = nc.dram_tensor("warm_in_d", [1, 128], FP8)
    warm_out_d = nc.dram_tensor("warm_out_d", [N_CORES, 1, 128], FP8,
                                addr_space="Shared")
    y1sh_a_d = nc.dram_tensor("y1sh_a_d", [P, JA * F], FP8)
    y1sh_b_d = nc.dram_tensor("y1sh_b_d", [P, (JC - JA) * F], FP8)
    y1all_a_d = nc.dram_tensor("y1all_a_d", [N_CORES, P, JA * F], FP8,
                               addr_space="Shared")
    y1all_b_d = nc.dram_tensor("y1all_b_d", [N_CORES, P, (JC - JA) * F], FP8,
                               addr_space="Shared")

    with tile.TileContext(nc) as tc:
        with (
            tc.tile_pool(name="persist", bufs=1) as pp,
            tc.tile_pool(name="work", bufs=2) as wp,
            tc.tile_pool(name="ps_s", bufs=1, space="PSUM") as ps_s,
            tc.tile_pool(name="ps_h", bufs=2, space="PSUM") as ps_h,
            tc.tile_pool(name="ps_b", bufs=1, space="PSUM") as ps_b,
            tc.tile_pool(name="ps_y", bufs=2, space="PSUM") as ps_y,
        ):
            # ---- persistent SBUF ----
            a_sb = pp.tile([P, KC, SHARD], FP8)
            xt_sb = pp.tile([P, G], FP8)
            xtsh_sb = pp.tile([P, SLOTS], BF16)
            y_sb = pp.tile([P, KC, F], FP8)
            y1loc = pp.tile([P, JC, F], FP8)
            x1T = pp.tile([P, SHARD], BF16)
            x2T = pp.tile([P, SHARD], BF16)
            z2sb = pp.tile([P, JC, FOUT], F32)
            zc = pp.tile([P, JC, FOUT], F32)
            expall = pp.tile([P, JC, FOUT], F32)
            outsb = pp.tile([P, JC, FOUT], F32)
            rmax = pp.tile([P, JC], F32)
            negmax = pp.tile([P, JC], F32)
            sumexp = pp.tile([P, JC], F32)
            lnsum = pp.tile([P, JC], F32)
            neglns = pp.tile([P, JC], F32)
            lin_w0_sb = pp.tile([F, F], FP8)
            lin_w1_sb = pp.tile([F, F], BF16)
            aggw0t_sb = pp.tile([F, F], BF16)
            aggw0b_sb = pp.tile([F, F], BF16)
            aggw1t_sb = pp.tile([F, F], BF16)
            aggw1b_sb = pp.tile([F, F], BF16)
            mp_w12_sb = pp.tile([F, FOUT], BF16)
            ones_mat = pp.tile([P, P], BF16)
            eps_sb = pp.tile([P, 1], F32)

            # warm-up collective: pays the cross-core rendezvous cost while
            # the a8 stream runs, so the real AllGathers launch promptly
            nc.gpsimd.collective_compute(
                "AllGather", mybir.AluOpType.bypass,
                replica_groups=[list(range(N_CORES))],
                ins=[warm_in_d[:]], outs=[warm_out_d[:]],
            )
            # ---- front loads: y0 + layer-0 deps first (DMA queues are FIFO,
            # ---- so xt must fully precede the big a8 stream) ----
            nc.sync.dma_start(lin_w0_sb[:], lin_w0_d[:])
            XH = G // 2
            nc.sync.dma_start(xt_sb[:, 0:XH], xt_d[:, 0:XH])
            nc.sync.dma_start(xt_sb[:, XH:G], xt_d[:, XH:G])
            nc.sync.dma_start(xtsh_sb[:], xtsh_d[:])
            nc.sync.dma_start(aggw0t_sb[:], agg_w0_d[0:F, :])
            nc.sync.dma_start(aggw0b_sb[:], agg_w0_d[F:2 * F, :])
            for s in range(KC // ASLAB):
                nc.sync.dma_start(a_sb[:, s * ASLAB:(s + 1) * ASLAB, :],
                                  a8_d[:, s * ASLAB:(s + 1) * ASLAB, :])
            nc.sync.dma_start(lin_w1_sb[:], lin_w1_d[:])
            nc.sync.dma_start(aggw1t_sb[:], agg_w1_d[0:F, :])
            nc.sync.dma_start(aggw1b_sb[:], agg_w1_d[F:2 * F, :])
            nc.sync.dma_start(mp_w12_sb[:], mp_w12_d[:])
            nc.gpsimd.memset(ones_mat[:], 1.0)
            nc.gpsimd.memset(eps_sb[:], 1e-24)
            nc.gpsimd.memset(y1loc[:, JC - 1, :], 0.0)
            nc.gpsimd.memset(rmax[:], 0.0)
            nc.gpsimd.memset(zc[:, :, :], 0.0)

            def y_batch(dst_tile, lhs_cols, w_sb, chunks4):
                """4 node-chunk matmuls into quarter-bank psum slots of one
                [P,512] tile + one wide vector relu into fp8 dst."""
                ps = ps_y.tile([P, 512], F32, tag="ps_y", name="ps_yb")
                for q, k in enumerate(chunks4):
                    nc.tensor.matmul(ps[:, q * F:(q + 1) * F],
                                     lhs_cols(k), w_sb[:],
                                     start=True, stop=True,
                                     skip_group_check=True)
                nc.vector.tensor_scalar_max(
                    dst_tile[:, chunks4[0]:chunks4[0] + 4, :], ps[:], 0.0)

            def scatter(ps_list, kps, first, last):
                """fp8 DoubleRow scatter matmuls: 3 psum banks accumulate
                aggr^T = y^T A for the n-chunks; kp-outer for DMA pacing."""
                for kp in kps:
                    for i, (n0, ns) in enumerate(NCHUNKS):
                        nc.tensor.matmul(
                            ps_list[i][:, 0:ns],
                            y_sb[:, 2 * kp:2 * kp + 2, :],
                            a_sb[:, 2 * kp:2 * kp + 2, n0:n0 + ns],
                            start=(kp == first), stop=(kp == last),
                            perf_mode=DR,
                        )

            def scatter_tail(ps_list, kps, first):
                """final scatter group, n-chunk outer: bank i stops as soon
                as its own pairs are done, so sage_update pipelines with the
                remaining banks' matmuls."""
                for i, (n0, ns) in enumerate(NCHUNKS):
                    for kp in kps:
                        nc.tensor.matmul(
                            ps_list[i][:, 0:ns],
                            y_sb[:, 2 * kp:2 * kp + 2, :],
                            a_sb[:, 2 * kp:2 * kp + 2, n0:n0 + ns],
                            start=(kp == first), stop=(kp == kps[-1]),
                            perf_mode=DR,
                        )

            # ---- layer 0: y0 = relu(x @ w0) interleaved with its scatter,
            # ---- paced by the a8 slab stream ----
            ps_l0 = [ps_s.tile([P, 512], F32, tag=f"s{i}", name=f"ps_l0_{i}")
                     for i in range(3)]
            for s in range(KC // ASLAB):
                for b in range(ASLAB // 4):
                    k0 = s * ASLAB + b * 4
                    y_batch(y_sb, lambda k: xt_sb[:, k * P:(k + 1) * P],
                            lin_w0_sb, list(range(k0, k0 + 4)))
                if s < KC // ASLAB - 1:
                    scatter(ps_l0, range(s * ASLAB // 2, (s + 1) * ASLAB // 2),
                            0, -1)
                else:
                    scatter_tail(ps_l0,
                                 list(range(s * ASLAB // 2, KP)), 0)

            def sage_update(ps_list, aggwt_sb, aggwb_sb, xout, chunks):
                """concat-linear + relu + L2 row norm (aggr already mean).
                Writes the normalized layer output into xout [P, SHARD] bf16.
                Emitted per chunk so later chunks can slip behind collective
                launches they don't feed."""
                for i in chunks:
                    n0, ns = NCHUNKS[i]
                    ps = ps_list[i]
                    aggrT = wp.tile([P, 512], BF16, tag="aggrT")
                    nc.vector.tensor_scalar_mul(aggrT[:, 0:ns], ps[:, 0:ns], 1.0)
                    ph = ps_h.tile([P, 512], F32, tag="ph")
                    nc.tensor.matmul(ph[:, 0:ns], aggwt_sb[:],
                                     xtsh_sb[:, n0:n0 + ns] if xout is x1T
                                     else x1T[:, n0:n0 + ns],
                                     start=True, stop=False)
                    nc.tensor.matmul(ph[:, 0:ns], aggwb_sb[:],
                                     aggrT[:, 0:ns], start=False, stop=True)
                    hT = wp.tile([P, 512], F32, tag="hT")
                    nc.vector.tensor_scalar_max(hT[:, 0:ns], ph[:, 0:ns], 0.0)
                    h2 = wp.tile([P, 512], BF16, tag="h2")
                    nc.vector.tensor_tensor(h2[:, 0:ns], hT[:, 0:ns],
                                            hT[:, 0:ns], mybir.AluOpType.mult)
                    pb = ps_b.tile([P, 512], F32, tag="pb")
                    nc.tensor.matmul(pb[:, 0:ns], ones_mat[:], h2[:, 0:ns],
                                     start=True, stop=True)
                    nrm = wp.tile([P, 512], F32, tag="nrm")
                    nc.scalar.activation(nrm[:, 0:ns], pb[:, 0:ns], AF.Sqrt,
                                         bias=eps_sb[:])
                    rinv = wp.tile([P, 512], F32, tag="rinv")
                    nc.vector.reciprocal_approx_fast(rinv[:, 0:ns],
                                                     nrm[:, 0:ns])
                    nc.vector.tensor_tensor(xout[:, n0:n0 + ns], hT[:, 0:ns],
                                            rinv[:, 0:ns],
                                            mybir.AluOpType.mult)

            # ---- y1 = relu(x1 @ lin_w1); batched, interleaved with the
            # ---- update chunks feeding it; chunk 2 and y1 j6-9 slip
            # ---- behind the first AllGather launch ----
            sage_update(ps_l0, aggw0t_sb, aggw0b_sb, x1T, [0])
            y_batch(y1loc, lambda j: x1T[:, j * P:(j + 1) * P],
                    lin_w1_sb, [0, 1, 2, 3])
            sage_update(ps_l0, aggw0t_sb, aggw0b_sb, x1T, [1])
            ps = ps_y.tile([P, 512], F32, tag="ps_y", name="ps_y45")
            for q, j in enumerate([4, 5]):
                nc.tensor.matmul(ps[:, q * F:(q + 1) * F],
                                 x1T[:, j * P:(j + 1) * P], lin_w1_sb[:],
                                 start=True, stop=True, skip_group_check=True)
            nc.vector.tensor_scalar_max(y1loc[:, 4:6, :], ps[:, 0:2 * F], 0.0)

            nc.sync.dma_start(y1sh_a_d[:], y1loc[:, 0:JA, :])
            nc.gpsimd.collective_compute(
                "AllGather", mybir.AluOpType.bypass,
                replica_groups=[list(range(N_CORES))],
                ins=[y1sh_a_d[:]], outs=[y1all_a_d[:]],
            )
            sage_update(ps_l0, aggw0t_sb, aggw0b_sb, x1T, [2])
            ps = ps_y.tile([P, 512], F32, tag="ps_y", name="ps_y69")
            for q, j in enumerate([6, 7, 8]):
                nc.tensor.matmul(ps[:, q * F:(q + 1) * F],
                                 x1T[:, j * P:(j + 1) * P], lin_w1_sb[:],
                                 start=True, stop=True, skip_group_check=True)
            jc9 = _jc(JC - 1)
            nc.tensor.matmul(ps[0:jc9, 3 * F:4 * F],
                             x1T[:, 9 * P:9 * P + jc9], lin_w1_sb[:],
                             start=True, stop=True, skip_group_check=True)
            nc.vector.tensor_scalar_max(y1loc[:, 6:9, :], ps[:, 0:3 * F], 0.0)
            nc.vector.tensor_scalar_max(y1loc[0:jc9, 9, :],
                                        ps[0:jc9, 3 * F:4 * F], 0.0)

            nc.sync.dma_start(y1sh_b_d[:], y1loc[:, JA:JC, :])
            nc.gpsimd.collective_compute(
                "AllGather", mybir.AluOpType.bypass,
                replica_groups=[list(range(N_CORES))],
                ins=[y1sh_b_d[:]], outs=[y1all_b_d[:]],
            )

            # reload gathered y1 into y_sb (chunk k = c*JC + j); half A lands
            # in pieces so its first scatter pairs start as early as possible
            ysb_v = y_sb[:, :, :].rearrange("p (c j) f -> p c (j f)", c=N_CORES)
            nc.sync.dma_start(ysb_v[:, :, 0:2 * F],
                              y1all_a_d[:, :, 0:2 * F].transpose([1, 0, 2]))
            nc.sync.dma_start(ysb_v[:, :, 2 * F:JA * F],
                              y1all_a_d[:, :, 2 * F:JA * F].transpose([1, 0, 2]))
            nc.sync.dma_start(ysb_v[:, :, JA * F:JC * F],
                              y1all_b_d[:].transpose([1, 0, 2]))

            # ---- layer 1: scatter half A first (overlaps AllGather B) ----
            kps_a1 = [c * (JC // 2) for c in range(N_CORES)]
            kps_a2 = [c * (JC // 2) + q for c in range(N_CORES) for q in (1, 2)]
            kps_b = [c * (JC // 2) + q for c in range(N_CORES)
                     for q in range(JA // 2, JC // 2)]
            ps_l1 = [ps_s.tile([P, 512], F32, tag=f"s{i}", name=f"ps_l1_{i}")
                     for i in range(3)]
            scatter(ps_l1, kps_a1, kps_a1[0], -1)
            scatter(ps_l1, kps_a2, -1, -1)
            scatter_tail(ps_l1, kps_b, -1)
            # preload Exp's act table before the softmax needs it (the
            # scalar engine is otherwise idle here)
            tblscr = wp.tile([P, 1], F32, tag="tblscr")
            nc.scalar.activation(tblscr[:], eps_sb[:], AF.Exp)
            sage_update(ps_l1, aggw1t_sb, aggw1b_sb, x2T, [0, 1])

            # ---- post_mp: z2 = x2 @ (mp_w1 @ mp_w2), node-major logits;
            # ---- max/shift per batch so softmax starts early; batch 0
            # ---- needs only x2 chunks 0-1 ----
            for g in range(2):  # z2 in two batches of <=8 chunks per bank
                j0, j1 = (0, 8) if g == 0 else (8, JC)
                if g == 1:
                    sage_update(ps_l1, aggw1t_sb, aggw1b_sb, x2T, [2])
                pz = ps_y.tile([P, 512], F32, tag="ps_y", name=f"ps_z{g}")
                for q, j in enumerate(range(j0, j1)):
                    jc = _jc(j)
                    nc.tensor.matmul(pz[0:jc, q * FOUT:(q + 1) * FOUT],
                                     x2T[:, j * P:j * P + jc], mp_w12_sb[:],
                                     start=True, stop=True,
                                     skip_group_check=True)
                nb = (j1 - j0) * FOUT
                nc.scalar.activation(
                    z2sb[:, j0:j1, :].rearrange("p j f -> p (j f)"),
                    pz[:, 0:nb], AF.Copy)
                nc.vector.tensor_reduce(rmax[:, j0:j1], z2sb[:, j0:j1, :],
                                        mybir.AxisListType.X,
                                        mybir.AluOpType.max)
                nc.vector.tensor_scalar_mul(negmax[:, j0:j1], rmax[:, j0:j1],
                                            -1.0)
                for j in range(j0, j1):
                    jc = _jc(j)
                    nc.vector.tensor_scalar_add(zc[0:jc, j, :],
                                                z2sb[0:jc, j, :],
                                                negmax[0:jc, j:j + 1])
                nc.scalar.activation(expall[:, j0:j1, :], zc[:, j0:j1, :],
                                     AF.Exp)
                nc.vector.tensor_reduce(sumexp[:, j0:j1],
                                        expall[:, j0:j1, :],
                                        mybir.AxisListType.X,
                                        mybir.AluOpType.add)

            # ---- log_softmax over classes, batched ----
            nc.scalar.activation(lnsum[:], sumexp[:], AF.Ln)
            nc.vector.tensor_scalar_mul(neglns[:], lnsum[:], -1.0)
            for j in range(JC):
                jc = _jc(j)
                nc.vector.tensor_scalar_add(outsb[0:jc, j, :], zc[0:jc, j, :],
                                            neglns[0:jc, j:j + 1])
            nfull = (JC - 1) * P  # 1152 nodes in full chunks
            nc.sync.dma_start(
                out_d[0:nfull, :].rearrange("(j p) f -> p j f", p=P),
                outsb[:, 0:JC - 1, :])
            nc.sync.dma_start(out_d[nfull:SHARD, :],
                              outsb[0:_jc(JC - 1), JC - 1, :])

    nc.compile()
    return nc


_NC = None


def _get_nc():
    global _NC
    if _NC is None:
        _NC = build()
    return _NC


def make_in_maps(inputs):
    x = np.asarray(inputs["x"], dtype=np.float32)
    ei = np.asarray(inputs["edge_index"])
    src = ei[0].astype(np.int64)
    dst = ei[1].astype(np.int64)

    cnt = np.bincount(dst, minlength=N_NODES).astype(np.float32)
    inv = (1.0 / np.maximum(cnt, 1.0)).astype(np.float32)

    # dense scatter-mean matrix: edge_count/deg(dst), padded src slots,
    # partition-major per core
    srcp = (src // SHARD) * SLOTS + (src % SHARD)
    flat = srcp * N_NODES + dst
    counts = np.bincount(flat, minlength=G * N_NODES)
    A = counts.reshape(G, N_NODES).astype(np.float32)
    del counts
    A *= inv[None, :]
    A8 = A.astype(NP_FP8).reshape(KC, P, N_NODES).transpose(1, 0, 2)
    del A

    # padded transposed features [128, 10240]
    xp = np.zeros((G, F), np.float32)
    for c in range(N_CORES):
        xp[c * SLOTS:c * SLOTS + SHARD] = x[c * SHARD:(c + 1) * SHARD]
    xt8 = np.ascontiguousarray(xp.T).astype(NP_FP8)
    xt16 = np.ascontiguousarray(xp.T).astype(NP_BF16)

    def w(name, dt=NP_BF16):
        return np.ascontiguousarray(
            np.asarray(inputs[name], np.float32)).astype(dt)

    w12 = np.asarray(inputs["mp_w1"], np.float32) @ np.asarray(
        inputs["mp_w2"], np.float32)
    common = {
        "xt": xt8,
        "lin_w0": w("lin_w0", NP_FP8), "lin_w1": w("lin_w1"),
        "agg_w0": w("agg_w0"), "agg_w1": w("agg_w1"),
        "mp_w12": np.ascontiguousarray(w12).astype(NP_BF16),
    }
    in_maps = []
    for c in range(N_CORES):
        lo, hi = c * SHARD, (c + 1) * SHARD
        in_maps.append({
            **common,
            "xt_sh": np.ascontiguousarray(xt16[:, c * SLOTS:(c + 1) * SLOTS]),
            "a8": np.ascontiguousarray(A8[:, :, lo:hi]),
        })
    return in_maps


def run(inputs, trace=False, **kwargs):
    nc = _get_nc()
    in_maps = make_in_maps(inputs)
    res = run_bass_kernel_spmd(nc, in_maps, core_ids=list(range(N_CORES)),
                               trace=trace, **kwargs)
    out = np.concatenate([res.results[c]["out"] for c in range(N_CORES)],
                         axis=0)
    return out.astype(np.float32), res


def kernel(**inputs):
    out, _ = run(inputs, trace=False)
    return out


# revision 49
# speedup vs baseline: 1.2233x; 1.2233x over previous
"""GraphSAGE 2-layer GNN + MLP head on 8 Trainium2 NeuronCores (v4).

Strategy (dst-sharded, dense-adjacency scatter, fp8 DoubleRow):
  - Destination nodes sharded across 8 cores; node index space padded to
    1280 slots/core (10240 global slots = 80 full 128-chunks) so every
    matmul chunk is full and fp8 DoubleRow pairs align.
  - Scatter-mean collapses to  aggr = (relu(X W))^T A_mean  with
    A_mean[src,dst] = edge_count/deg(dst) in fp8 e4m3 (mean folded in
    host-side), resident in SBUF and reused by both layers.
    Scatter matmuls run in fp8 DoubleRow (K=256 per instruction).
  - A is stored partition-major in DRAM ([128, 80, 1250]); 10 slab
    dma_starts with 12.5KB/partition descriptors saturate HBM; y0 and
    the layer-0 scatter are emitted interleaved per slab.
  - y0/y1/z2 copy/activation work is batched 4 chunks per PSUM bank
    (512B quarter-bank matmul outputs, one wide relu per batch) to avoid
    serializing on per-chunk scalar ops.
  - Inter-layer AllGather of fp8 y1 split in two (6+4 chunks): the
    second collective overlaps the first half's scatter matmuls.
  - Row L2-norm: ones-matmul partition reduction, scalar Sqrt, DVE fast
    reciprocal - all partition-parallel.
  - log_softmax: second post_mp matmul emits node-major logits directly;
    single big Exp + tensor_reduce keeps act-table loads at 2.
"""

import numpy as np
import ml_dtypes

import concourse.bacc as bacc
import concourse.mybir as mybir
from concourse import tile
from concourse.bass_utils import run_bass_kernel_spmd

N_NODES = 10000
N_CORES = 8
SHARD = N_NODES // N_CORES   # 1250 real dst nodes per core
P = 128
JC = 10                      # local 128-chunks per core (1280 slots)
SLOTS = JC * P               # 1280 padded slots per core
G = N_CORES * SLOTS          # 10240 padded global slots
KC = G // P                  # 80 src chunks
KP = KC // 2                 # 40 DoubleRow pairs
F = 128
FOUT = 64
NCHUNKS = [(0, 512), (512, 512), (1024, SHARD - 1024)]
JA = 6                       # allgather half A: local chunks 0..5
ASLAB = 8                    # a8 chunks per dma slab

FP8 = mybir.dt.float8e4
BF16 = mybir.dt.bfloat16
F32 = mybir.dt.float32
DR = mybir.MatmulPerfMode.DoubleRow
AF = mybir.ActivationFunctionType

NP_FP8 = ml_dtypes.float8_e4m3
NP_BF16 = ml_dtypes.bfloat16


def _jc(j):
    """real node count in local chunk j (last chunk is partial: 98)."""
    return min(P, SHARD - j * P)


def build():
    nc = bacc.Bacc("TRN2", target_bir_lowering=False, debug=False,
                   num_devices=N_CORES)

    # ---- external I/O ----
    xt_d = nc.declare_dram_parameter("xt", [P, G], FP8, isOutput=False)
    xtsh_d = nc.declare_dram_parameter("xt_sh", [P, SLOTS], BF16, isOutput=False)
    a8_d = nc.declare_dram_parameter("a8", [P, KC, SHARD], FP8, isOutput=False)
    lin_w0_d = nc.declare_dram_parameter("lin_w0", [F, F], FP8, isOutput=False)
    lin_w1_d = nc.declare_dram_parameter("lin_w1", [F, F], BF16, isOutput=False)
    agg_w0_d = nc.declare_dram_parameter("agg_w0", [2 * F, F], BF16, isOutput=False)
    agg_w1_d = nc.declare_dram_parameter("agg_w1", [2 * F, F], BF16, isOutput=False)
    mp_w12_d = nc.declare_dram_parameter("mp_w12", [F, FOUT], BF16,
                                         isOutput=False)
    out_d = nc.declare_dram_parameter("out", [SHARD, FOUT], F32, isOutput=True)

    # internal DRAM for the split inter-layer AllGather
    warm_in_d = nc.dram_tensor("warm_in_d", [1, 128], FP8)
    warm_out_d = nc.dram_tensor("warm_out_d", [N_CORES, 1, 128], FP8,
                                addr_space="Shared")
    y1sh_a_d = nc.dram_tensor("y1sh_a_d", [P, JA * F], FP8)
    y1sh_b_d = nc.dram_tensor("y1sh_b_d", [P, (JC - JA) * F], FP8)
    y1all_a_d = nc.dram_tensor("y1all_a_d", [N_CORES, P, JA * F], FP8,
                               addr_space="Shared")
    y1all_b_d = nc.dram_tensor("y1all_b_d", [N_CORES, P, (JC - JA) * F], FP8,
                               addr_space="Shared")

    with tile.TileContext(nc) as tc:
        with (
            tc.tile_pool(name="persist", bufs=1) as pp,
            tc.tile_pool(name="work", bufs=2) as wp,
            tc.tile_pool(name="ps_s", bufs=1, space="PSUM") as ps_s,
            tc.tile_pool(name="ps_h", bufs=2, space="PSUM") as ps_h,
            tc.tile_pool(name="ps_b", bufs=1, space="PSUM") as ps_b,
            tc.tile_pool(name="ps_y", bufs=2, space="PSUM") as ps_y,
        ):
            # ---- persistent SBUF ----
            a_sb = pp.tile([P, KC, SHARD], FP8)
            xt_sb = pp.tile([P, G], FP8)
            xtsh_sb = pp.tile([P, SLOTS], BF16)
            y_sb = pp.tile([P, KC, F], FP8)
            y1loc = pp.tile([P, JC, F], FP8)
            x1T = pp.tile([P, SHARD], BF16)
            x2T = pp.tile([P, SHARD], BF16)
            z2sb = pp.tile([P, JC, FOUT], F32)
            zc = pp.tile([P, JC, FOUT], F32)
            expall = pp.tile([P, JC, FOUT], F32)
            outsb = pp.tile([P, JC, FOUT], F32)
            rmax = pp.tile([P, JC], F32)
            negmax = pp.tile([P, JC], F32)
            sumexp = pp.tile([P, JC], F32)
            lnsum = pp.tile([P, JC], F32)
            neglns = pp.tile([P, JC], F32)
            lin_w0_sb = pp.tile([F, F], FP8)
            lin_w1_sb = pp.tile([F, F], BF16)
            aggw0t_sb = pp.tile([F, F], BF16)
            aggw0b_sb = pp.tile([F, F], BF16)
            aggw1t_sb = pp.tile([F, F], BF16)
            aggw1b_sb = pp.tile([F, F], BF16)
            mp_w12_sb = pp.tile([F, FOUT], BF16)
            ones_mat = pp.tile([P, P], BF16)
            eps_sb = pp.tile([P, 1], F32)

            # warm-up collective: pays the cross-core rendezvous cost while
            # the a8 stream runs, so the real AllGathers launch promptly
            nc.gpsimd.collective_compute(
                "AllGather", mybir.AluOpType.bypass,
                replica_groups=[list(range(N_CORES))],
                ins=[warm_in_d[:]], outs=[warm_out_d[:]],
            )
            # ---- front loads: y0 + layer-0 deps first (DMA queues are FIFO,
            # ---- so xt must fully precede the big a8 stream) ----
            nc.sync.dma_start(lin_w0_sb[:], lin_w0_d[:])
            XH = G // 2
            nc.sync.dma_start(xt_sb[:, 0:XH], xt_d[:, 0:XH])
            nc.sync.dma_start(xt_sb[:, XH:G], xt_d[:, XH:G])
            nc.sync.dma_start(xtsh_sb[:], xtsh_d[:])
            nc.sync.dma_start(aggw0t_sb[:], agg_w0_d[0:F, :])
            nc.sync.dma_start(aggw0b_sb[:], agg_w0_d[F:2 * F, :])
            for s in range(KC // ASLAB):
                nc.sync.dma_start(a_sb[:, s * ASLAB:(s + 1) * ASLAB, :],
                                  a8_d[:, s * ASLAB:(s + 1) * ASLAB, :])
            nc.sync.dma_start(lin_w1_sb[:], lin_w1_d[:])
            nc.sync.dma_start(aggw1t_sb[:], agg_w1_d[0:F, :])
            nc.sync.dma_start(aggw1b_sb[:], agg_w1_d[F:2 * F, :])
            nc.sync.dma_start(mp_w12_sb[:], mp_w12_d[:])
            nc.gpsimd.memset(ones_mat[:], 1.0)
            nc.gpsimd.memset(eps_sb[:], 1e-24)
            nc.gpsimd.memset(y1loc[:, JC - 1, :], 0.0)
            nc.gpsimd.memset(rmax[:], 0.0)
            nc.gpsimd.memset(zc[:, :, :], 0.0)

            def y_batch(dst_tile, lhs_cols, w_sb, chunks4):
                """4 node-chunk matmuls into quarter-bank psum slots of one
                [P,512] tile + one wide vector relu into fp8 dst."""
                ps = ps_y.tile([P, 512], F32, tag="ps_y", name="ps_yb")
                for q, k in enumerate(chunks4):
                    nc.tensor.matmul(ps[:, q * F:(q + 1) * F],
                                     lhs_cols(k), w_sb[:],
                                     start=True, stop=True,
                                     skip_group_check=True)
                nc.vector.tensor_scalar_max(
                    dst_tile[:, chunks4[0]:chunks4[0] + 4, :], ps[:], 0.0)

            def scatter(ps_list, kps, first, last):
                """fp8 DoubleRow scatter matmuls: 3 psum banks accumulate
                aggr^T = y^T A for the n-chunks; kp-outer for DMA pacing."""
                for kp in kps:
                    for i, (n0, ns) in enumerate(NCHUNKS):
                        nc.tensor.matmul(
                            ps_list[i][:, 0:ns],
                            y_sb[:, 2 * kp:2 * kp + 2, :],
                            a_sb[:, 2 * kp:2 * kp + 2, n0:n0 + ns],
                            start=(kp == first), stop=(kp == last),
                            perf_mode=DR,
                        )

            def scatter_tail(ps_list, kps, first):
                """final scatter group, n-chunk outer: bank i stops as soon
                as its own pairs are done, so sage_update pipelines with the
                remaining banks' matmuls."""
                for i, (n0, ns) in enumerate(NCHUNKS):
                    for kp in kps:
                        nc.tensor.matmul(
                            ps_list[i][:, 0:ns],
                            y_sb[:, 2 * kp:2 * kp + 2, :],
                            a_sb[:, 2 * kp:2 * kp + 2, n0:n0 + ns],
                            start=(kp == first), stop=(kp == kps[-1]),
                            perf_mode=DR,
                        )

            # ---- layer 0: y0 = relu(x @ w0) interleaved with its scatter,
            # ---- paced by the a8 slab stream ----
            ps_l0 = [ps_s.tile([P, 512], F32, tag=f"s{i}", name=f"ps_l0_{i}")
                     for i in range(3)]
            for s in range(KC // ASLAB):
                for b in range(ASLAB // 4):
                    k0 = s * ASLAB + b * 4
                    y_batch(y_sb, lambda k: xt_sb[:, k * P:(k + 1) * P],
                            lin_w0_sb, list(range(k0, k0 + 4)))
                if s < KC // ASLAB - 1:
                    scatter(ps_l0, range(s * ASLAB // 2, (s + 1) * ASLAB // 2),
                            0, -1)
                else:
                    scatter_tail(ps_l0,
                                 list(range(s * ASLAB // 2, KP)), 0)

            def sage_update(ps_list, aggwt_sb, aggwb_sb, xout, chunks):
                """concat-linear + relu + L2 row norm (aggr already mean).
                Writes the normalized layer output into xout [P, SHARD] bf16.
                Emitted per chunk so later chunks can slip behind collective
                launches they don't feed."""
                for i in chunks:
                    n0, ns = NCHUNKS[i]
                    ps = ps_list[i]
                    aggrT = wp.tile([P, 512], BF16, tag="aggrT")
                    nc.vector.tensor_scalar_mul(aggrT[:, 0:ns], ps[:, 0:ns], 1.0)
                    ph = ps_h.tile([P, 512], F32, tag="ph")
                    nc.tensor.matmul(ph[:, 0:ns], aggwt_sb[:],
                                     xtsh_sb[:, n0:n0 + ns] if xout is x1T
                                     else x1T[:, n0:n0 + ns],
                                     start=True, stop=False)
                    nc.tensor.matmul(ph[:, 0:ns], aggwb_sb[:],
                                     aggrT[:, 0:ns], start=False, stop=True)
                    hT = wp.tile([P, 512], F32, tag="hT")
                    nc.vector.tensor_scalar_max(hT[:, 0:ns], ph[:, 0:ns], 0.0)
                    h2 = wp.tile([P, 512], BF16, tag="h2")
                    nc.vector.tensor_tensor(h2[:, 0:ns], hT[:, 0:ns],
                                            hT[:, 0:ns], mybir.AluOpType.mult)
                    pb = ps_b.tile([P, 512], F32, tag="pb")
                    nc.tensor.matmul(pb[:, 0:ns], ones_mat[:], h2[:, 0:ns],
                                     start=True, stop=True)
                    nrm = wp.tile([P, 512], F32, tag="nrm")
                    nc.scalar.activation(nrm[:, 0:ns], pb[:, 0:ns], AF.Sqrt,
                                         bias=eps_sb[:])
                    rinv = wp.tile([P, 512], F32, tag="rinv")
                    nc.vector.reciprocal_approx_fast(rinv[:, 0:ns],
                                                     nrm[:, 0:ns])
                    nc.vector.tensor_tensor(xout[:, n0:n0 + ns], hT[:, 0:ns],
                                            rinv[:, 0:ns],
                                            mybir.AluOpType.mult)

            # ---- y1 = relu(x1 @ lin_w1); batched, interleaved with the
            # ---- update chunks feeding it; chunk 2 and y1 j6-9 slip
            # ---- behind the first AllGather launch ----
            sage_update(ps_l0, aggw0t_sb, aggw0b_sb, x1T, [0])
            y_batch(y1loc, lambda j: x1T[:, j * P:(j + 1) * P],
                    lin_w1_sb, [0, 1, 2, 3])
            sage_update(ps_l0, aggw0t_sb, aggw0b_sb, x1T, [1])
            ps = ps_y.tile([P, 512], F32, tag="ps_y", name="ps_y45")
            for q, j in enumerate([4, 5]):
                nc.tensor.matmul(ps[:, q * F:(q + 1) * F],
                                 x1T[:, j * P:(j + 1) * P], lin_w1_sb[:],
                                 start=True, stop=True, skip_group_check=True)
            nc.vector.tensor_scalar_max(y1loc[:, 4:6, :], ps[:, 0:2 * F], 0.0)

            nc.sync.dma_start(y1sh_a_d[:], y1loc[:, 0:JA, :])
            nc.gpsimd.collective_compute(
                "AllGather", mybir.AluOpType.bypass,
                replica_groups=[list(range(N_CORES))],
                ins=[y1sh_a_d[:]], outs=[y1all_a_d[:]],
            )
            sage_update(ps_l0, aggw0t_sb, aggw0b_sb, x1T, [2])
            ps = ps_y.tile([P, 512], F32, tag="ps_y", name="ps_y69")
            for q, j in enumerate([6, 7, 8]):
                nc.tensor.matmul(ps[:, q * F:(q + 1) * F],
                                 x1T[:, j * P:(j + 1) * P], lin_w1_sb[:],
                                 start=True, stop=True, skip_group_check=True)
            jc9 = _jc(JC - 1)
            nc.tensor.matmul(ps[0:jc9, 3 * F:4 * F],
                             x1T[:, 9 * P:9 * P + jc9], lin_w1_sb[:],
                             start=True, stop=True, skip_group_check=True)
            nc.vector.tensor_scalar_max(y1loc[:, 6:9, :], ps[:, 0:3 * F], 0.0)
            nc.vector.tensor_scalar_max(y1loc[0:jc9, 9, :],
                                        ps[0:jc9, 3 * F:4 * F], 0.0)

            nc.sync.dma_start(y1sh_b_d[:], y1loc[:, JA:JC, :])
            nc.gpsimd.collective_compute(
                "AllGather", mybir.AluOpType.bypass,
                replica_groups=[list(range(N_CORES))],
                ins=[y1sh_b_d[:]], outs=[y1all_b_d[:]],
            )

            # reload gathered y1 into y_sb (chunk k = c*JC + j); half A lands
            # in pieces so its first scatter pairs start as early as possible
            ysb_v = y_sb[:, :, :].rearrange("p (c j) f -> p c (j f)", c=N_CORES)
            nc.sync.dma_start(ysb_v[:, :, 0:2 * F],
                              y1all_a_d[:, :, 0:2 * F].transpose([1, 0, 2]))
            nc.sync.dma_start(ysb_v[:, :, 2 * F:JA * F],
                              y1all_a_d[:, :, 2 * F:JA * F].transpose([1, 0, 2]))
            nc.sync.dma_start(ysb_v[:, :, JA * F:JC * F],
                              y1all_b_d[:].transpose([1, 0, 2]))

            # ---- layer 1: scatter half A first (overlaps AllGather B) ----
            kps_a1 = [c * (JC // 2) for c in range(N_CORES)]
            kps_a2 = [c * (JC // 2) + q for c in range(N_CORES) for q in (1, 2)]
            kps_b = [c * (JC // 2) + q for c in range(N_CORES)
                     for q in range(JA // 2, JC // 2)]
            ps_l1 = [ps_s.tile([P, 512], F32, tag=f"s{i}", name=f"ps_l1_{i}")
                     for i in range(3)]
            scatter(ps_l1, kps_a1, kps_a1[0], -1)
            scatter(ps_l1, kps_a2, -1, -1)
            scatter_tail(ps_l1, kps_b, -1)
            # preload Exp's act table before the softmax needs it (the
            # scalar engine is otherwise idle here)
            tblscr = wp.tile([P, 1], F32, tag="tblscr")
            nc.scalar.activation(tblscr[:], eps_sb[:], AF.Exp)
            sage_update(ps_l1, aggw1t_sb, aggw1b_sb, x2T, [0, 1])

            # ---- post_mp: z2 = x2 @ (mp_w1 @ mp_w2), node-major logits;
            # ---- max/shift per batch so softmax starts early; batch 0
            # ---- needs only x2 chunks 0-1 ----
            for g in range(2):  # z2 in two batches of <=8 chunks per bank
                j0, j1 = (0, 8) if g == 0 else (8, JC)
                if g == 1:
                    sage_update(ps_l1, aggw1t_sb, aggw1b_sb, x2T, [2])
                pz = ps_y.tile([P, 512], F32, tag="ps_y", name=f"ps_z{g}")
                for q, j in enumerate(range(j0, j1)):
                    jc = _jc(j)
                    nc.tensor.matmul(pz[0:jc, q * FOUT:(q + 1) * FOUT],
                                     x2T[:, j * P:j * P + jc], mp_w12_sb[:],
                                     start=True, stop=True,
                                     skip_group_check=True)
                nb = (j1 - j0) * FOUT
                nc.scalar.activation(
                    z2sb[:, j0:j1, :].rearrange("p j f -> p (j f)"),
                    pz[:, 0:nb], AF.Copy)
                nc.vector.tensor_reduce(rmax[:, j0:j1], z2sb[:, j0:j1, :],
                                        mybir.AxisListType.X,
                                        mybir.AluOpType.max)
                nc.vector.tensor_scalar_mul(negmax[:, j0:j1], rmax[:, j0:j1],
                                            -1.0)
                nc.vector.tensor_tensor(
                    zc[:, j0:j1, :], z2sb[:, j0:j1, :],
                    negmax[:, j0:j1].unsqueeze(2).broadcast_to(
                        [P, j1 - j0, FOUT]),
                    mybir.AluOpType.add)
                nc.scalar.activation(expall[:, j0:j1, :], zc[:, j0:j1, :],
                                     AF.Exp)
                nc.vector.tensor_reduce(sumexp[:, j0:j1],
                                        expall[:, j0:j1, :],
                                        mybir.AxisListType.X,
                                        mybir.AluOpType.add)

            # ---- log_softmax over classes, batched ----
            nc.scalar.activation(lnsum[:], sumexp[:], AF.Ln)
            nc.vector.tensor_scalar_mul(neglns[:], lnsum[:], -1.0)
            nc.vector.tensor_tensor(
                outsb[:, :, :], zc[:, :, :],
                neglns[:].unsqueeze(2).broadcast_to([P, JC, FOUT]),
                mybir.AluOpType.add)
            nfull = (JC - 1) * P  # 1152 nodes in full chunks
            nc.sync.dma_start(
                out_d[0:nfull, :].rearrange("(j p) f -> p j f", p=P),
                outsb[:, 0:JC - 1, :])
            nc.sync.dma_start(out_d[nfull:SHARD, :],
                              outsb[0:_jc(JC - 1), JC - 1, :])

    nc.compile()
    return nc


_NC = None


def _get_nc():
    global _NC
    if _NC is None:
        _NC = build()
    return _NC


def make_in_maps(inputs):
    x = np.asarray(inputs["x"], dtype=np.float32)
    ei = np.asarray(inputs["edge_index"])
    src = ei[0].astype(np.int64)
    dst = ei[1].astype(np.int64)

    cnt = np.bincount(dst, minlength=N_NODES).astype(np.float32)
    inv = (1.0 / np.maximum(cnt, 1.0)).astype(np.float32)

    # dense scatter-mean matrix: edge_count/deg(dst), padded src slots,
    # partition-major per core
    srcp = (src // SHARD) * SLOTS + (src % SHARD)
    flat = srcp * N_NODES + dst
    counts = np.bincount(flat, minlength=G * N_NODES)
    A = counts.reshape(G, N_NODES).astype(np.float32)
    del counts
    A *= inv[None, :]
    A8 = A.astype(NP_FP8).reshape(KC, P, N_NODES).transpose(1, 0, 2)
    del A

    # padded transposed features [128, 10240]
    xp = np.zeros((G, F), np.float32)
    for c in range(N_CORES):
        xp[c * SLOTS:c * SLOTS + SHARD] = x[c * SHARD:(c + 1) * SHARD]
    xt8 = np.ascontiguousarray(xp.T).astype(NP_FP8)
    xt16 = np.ascontiguousarray(xp.T).astype(NP_BF16)

    def w(name, dt=NP_BF16):
        return np.ascontiguousarray(
            np.asarray(inputs[name], np.float32)).astype(dt)

    w12 = np.asarray(inputs["mp_w1"], np.float32) @ np.asarray(
        inputs["mp_w2"], np.float32)
    common = {
        "xt": xt8,
        "lin_w0": w("lin_w0", NP_FP8), "lin_w1": w("lin_w1"),
        "agg_w0": w("agg_w0"), "agg_w1": w("agg_w1"),
        "mp_w12": np.ascontiguousarray(w12).astype(NP_BF16),
    }
    in_maps = []
    for c in range(N_CORES):
        lo, hi = c * SHARD, (c + 1) * SHARD
        in_maps.append({
            **common,
            "xt_sh": np.ascontiguousarray(xt16[:, c * SLOTS:(c + 1) * SLOTS]),
            "a8": np.ascontiguousarray(A8[:, :, lo:hi]),
        })
    return in_maps


def run(inputs, trace=False, **kwargs):
    nc = _get_nc()
    in_maps = make_in_maps(inputs)
    res = run_bass_kernel_spmd(nc, in_maps, core_ids=list(range(N_CORES)),
                               trace=trace, **kwargs)
    out = np.concatenate([res.results[c]["out"] for c in range(N_CORES)],
                         axis=0)
    return out.astype(np.float32), res


def kernel(**inputs):
    out, _ = run(inputs, trace=False)
    return out
